# revision 11
# baseline (speedup 1.0000x reference)
"""LoRA linear y = x @ (B@A).T computed low-rank: y = (x @ A.T) @ B.T.

Sharding: data-parallel over tokens (B*S = 16384) across 8 NeuronCores,
2048 tokens/core; lora_A / lora_B replicated (tiny). No collectives.

All device I/O in bf16 (rel err ~3.5e-3, gate 2e-2): halves HBM traffic vs
f32 (64 MB -> 32 MB per core; ~358 GB/s/NC => ~90us floor). Host
pre-transposes x into xT chunk layout so the kernel needs NO on-device
transpose.

mm1 is 4x column-tiled on the PE array (tile_position=(0,32j)): the four
512-token slabs of tT stream concurrently through disjoint 32-column strips,
cutting mm1 PE time ~4x (~16K cycles). Each slab's tT lands at PSUM
partitions 32j..32j+16 -- exactly where mm2 wants its K=16 operands, so
mm2 runs row-positioned (tile_position=(32j,0)) against a B.T replicated at
partition offsets 0/32/64/96, and no cross-partition shuffle is ever needed.
With mm1 nearly free, PE work is mm2-drain-bound (~65K cycles) and fits
under the store phase even if the HAM clock gate throttles the PE cold.

DRAM layouts (per core, bf16):
  xtd [8ld*128p, 4jc*2048t] : xtd[ld,p,jc,t] = x[tok0+t, (ld*4+jc)*128+p]
  atp [128, 32*16]          : atp[p, c*16+r] = A[r, c*128+p]
  btr [128, 4096]           : btr[32j+r, d] = B.T[r, d]  (x4 replicated)
  ys  [8sd*128p, 2jj*4096]  : ys[sd,p,jj,:] = y[tok0+sd*256+jj*128+p, :]

Phase 1 (read-bound): 8x 2MB loads (sync ring); mm1 accumulates
tT[16,512] x4 slabs (K=128, x32) in 4 PSUM banks. tT -> SBUF bf16.
Phase 2 (write-bound): mm2 y[128,512] (K=16) pairs into [128,2,512] PSUM
tiles, one [128,1024] f32->bf16 copy per pair (DVE/ACT alternating),
8x 2MB stores (scalar ring).
"""

import os
import numpy as np
import ml_dtypes

import concourse.bass as bass
import concourse.mybir as mybir
from concourse.tile import TileContext
from concourse.bass_utils import run_bass_kernel_spmd

N_CORES = 8
B, S, D_IN, D_OUT, R = 4, 4096, 4096, 4096, 16
TOK = B * S
TPC = TOK // N_CORES   # tokens per core: 2048
NC_DIN = D_IN // 128   # 32 din chunks
NLD = 8                # x loads per core (4 chunks each, 2 MB)
NSD = 8                # y stores per core (256 tokens each, 2 MB)
F32 = mybir.dt.float32
BF16 = mybir.dt.bfloat16
NPBF16 = np.dtype(ml_dtypes.bfloat16)


def _split_drain_waits(nc):
    """This walrus build rejects instructions carrying >1 sem wait; hoist
    extra waits onto preceding single-wait NoOps on the same engine."""
    f = nc.m.functions[0]

    def fix_bb(bb):
        insts = getattr(bb, "instructions", None)
        if insts:
            new = []
            for inst in insts:
                si = inst.sync_info
                if si is not None and si.on_wait is not None and len(si.on_wait) > 1:
                    waits = list(si.on_wait)
                    for w in waits[:-1]:
                        d = mybir.InstNoOp(
                            name=nc.get_next_instruction_name(), ins=[], outs=[]
                        )
                        d.engine = inst.engine
                        d.sync_info = mybir.SyncInfo(on_wait=[w], on_update=[])
                        new.append(d)
                    si.on_wait = [waits[-1]]
                    inst.sync_info = si
                new.append(inst)
            bb.instructions[:] = new
        for sub in getattr(bb, "blocks", []) or []:
            fix_bb(sub)

    for blk in f.blocks:
        fix_bb(blk)


def _build():
    nc = bass.Bass("TRN2", target_bir_lowering=False, debug=False, num_devices=N_CORES)
    xtd = nc.declare_dram_parameter("xtd", [NLD * 128, 4 * TPC], BF16, isOutput=False)
    atp = nc.declare_dram_parameter("atp", [128, NC_DIN * R], BF16, isOutput=False)
    btr = nc.declare_dram_parameter("btr", [128, D_OUT], BF16, isOutput=False)
    ys = nc.declare_dram_parameter("ys", [NSD * 128, 2 * D_OUT], BF16, isOutput=True)

    with TileContext(nc) as tc:
        with (
            tc.tile_pool(name="const", bufs=1) as cpool,
            tc.tile_pool(name="x", bufs=int(os.environ.get("XB", "3"))) as xpool,
            tc.tile_pool(name="t", bufs=1) as tpool,
            tc.tile_pool(name="y", bufs=int(os.environ.get("YB", "3"))) as ypool,
            tc.tile_pool(name="t_ps", bufs=1, space="PSUM") as tpsum,
            tc.tile_pool(name="y_ps", bufs=int(os.environ.get("YPB", "2")), space="PSUM") as ypsum,
        ):
            at_sb = cpool.tile([128, NC_DIN * R], BF16)
            nc.scalar.dma_start(out=at_sb[:], in_=atp[:])
            bt_sb = cpool.tile([128, D_OUT], BF16)
            nc.scalar.dma_start(out=bt_sb[:], in_=btr[:])

            # phase 1: tT[16, 512] x 4 col-tiled slabs, one PSUM bank each
            tps = [tpsum.tile([128, 512], F32, name=f"tps{j}") for j in range(4)]
            for ld in range(NLD):
                xt = xpool.tile([128, 4, TPC], BF16)
                nc.sync.dma_start(out=xt[:], in_=xtd[ld * 128 : (ld + 1) * 128, :])
                for jc in range(4):
                    c = ld * 4 + jc
                    for j in range(4):
                        nc.tensor.matmul(
                            tps[j][32 * j : 32 * j + R, :],
                            at_sb[:, c * R : (c + 1) * R],
                            xt[:, jc, j * 512 : (j + 1) * 512],
                            start=(c == 0),
                            stop=(c == NC_DIN - 1),
                            tile_position=(0, 32 * j),
                        )

            t_sb = tpool.tile([128, 512], BF16)
            for j in range(4):
                if j % 2 == 0:
                    nc.vector.tensor_copy(
                        out=t_sb[32 * j : 32 * j + R, :],
                        in_=tps[j][32 * j : 32 * j + R, :],
                    )
                else:
                    nc.scalar.activation(
                        out=t_sb[32 * j : 32 * j + R, :],
                        in_=tps[j][32 * j : 32 * j + R, :],
                        func=mybir.ActivationFunctionType.Identity,
                    )

            # phase 2: mm2 + downcast copies + 2MB stores
            u = 0
            for sd in range(NSD):
                y_sb = ypool.tile([128, 2, D_OUT], BF16)
                for jj in range(2):
                    g = sd * 2 + jj
                    j, gg = g // 4, g % 4
                    for nbp in range(4):
                        yp = ypsum.tile([128, 2, 512], F32)
                        for k in range(2):
                            nc.tensor.matmul(
                                yp[:, k, :],
                                t_sb[32 * j : 32 * j + R, gg * 128 : (gg + 1) * 128],
                                bt_sb[32 * j : 32 * j + R, (nbp * 2 + k) * 512 : (nbp * 2 + k + 1) * 512],
                                start=True,
                                stop=True,
                                tile_position=(32 * j, 0),
                            )
                        if u % 2 == 0:
                            nc.vector.tensor_copy(
                                out=y_sb[:, jj, nbp * 1024 : (nbp + 1) * 1024],
                                in_=yp[:],
                            )
                        else:
                            nc.scalar.activation(
                                out=y_sb[:, jj, nbp * 1024 : (nbp + 1) * 1024],
                                in_=yp[:],
                                func=mybir.ActivationFunctionType.Identity,
                            )
                        u += 1
                nc.scalar.dma_start(
                    out=ys[sd * 128 : (sd + 1) * 128, :], in_=y_sb[:]
                )

    _split_drain_waits(nc)
    return nc


_NC = None


def _get_nc():
    global _NC
    if _NC is None:
        _NC = _build()
    return _NC


def _prep_inputs(x, lora_A, lora_B):
    x_flat = np.asarray(x, dtype=np.float32).reshape(TOK, D_IN)
    xb16 = x_flat.astype(NPBF16).view(np.uint16)
    A = np.asarray(lora_A, dtype=np.float32)
    Bm = np.asarray(lora_B, dtype=np.float32)
    xtds = []
    for i in range(N_CORES):
        # [t, ld, jc, p] -> [ld, p, jc, t]
        xc = xb16[i * TPC : (i + 1) * TPC].reshape(TPC, NLD, 4, 128)
        xtd = (
            np.ascontiguousarray(xc.transpose(1, 3, 2, 0))
            .reshape(NLD * 128, 4 * TPC)
            .view(NPBF16)
        )
        xtds.append(xtd)
    # atp[p, c*R + r] = A[r, c*128 + p]
    atp = np.ascontiguousarray(
        A.T.reshape(NC_DIN, 128, R).transpose(1, 0, 2).reshape(128, NC_DIN * R)
    ).astype(NPBF16)
    # btr[32j + r, :] = B.T[r, :], replicated at partition offsets 0/32/64/96
    btv = np.ascontiguousarray(Bm.T).astype(NPBF16)
    btrm = np.zeros((128, D_OUT), dtype=NPBF16)
    for j in range(4):
        btrm[32 * j : 32 * j + R] = btv
    return xtds, atp, btrm


def kernel(x, lora_A, lora_B, _trace=False, _trace_kwargs=None):
    nc = _get_nc()
    xtds, atp, btrm = _prep_inputs(x, lora_A, lora_B)
    in_maps = [{"xtd": xtds[i], "atp": atp, "btr": btrm} for i in range(N_CORES)]
    res = run_bass_kernel_spmd(
        nc, in_maps, list(range(N_CORES)), trace=_trace, **(_trace_kwargs or {})
    )
    out = np.empty((TOK, D_OUT), dtype=np.float32)
    for i in range(N_CORES):
        # ys [sd, p, jj, d] -> tokens sd*256 + jj*128 + p
        u = (
            np.asarray(res.results[i]["ys"])
            .view(np.uint16)
            .reshape(NSD, 128, 2, D_OUT)
            .transpose(0, 2, 1, 3)
        )
        out[i * TPC : (i + 1) * TPC] = (
            np.ascontiguousarray(u).reshape(TPC, D_OUT).view(NPBF16).astype(np.float32)
        )
    out = out.reshape(B, S, D_OUT)
    if _trace:
        return out, res
    return out
